# revision 12
# baseline (speedup 1.0000x reference)
"""Trainium2 Bass kernel for nn_BertGTHead_37177236914708 (BertGT pooling head).

Full-input contract: kernel(**inputs) takes the complete (unsharded) numpy
inputs and returns the full [B, 1+G] float32 output.

Strategy (data-parallel over batch, 2 examples per NeuronCore, 8 cores):
  - Host compacts each example's valid rows (base_mask=1, ~50% of S) into a
    dense order-preserving fp16 array, zero-padded to capacity C. The device
    streams C rows/example with no mask ops (zero-padding is exact for sums
    and absorbed by the final max(.,0)).
  - Stream: tapered chunks of (4,4,4,3,2,1) 128-row j-groups per example
    ([128, j*H] tiles), ex0 on the sync HWDGE ring, ex1 on the scalar ring.
    Tapering keeps VectorE fed as chunks arrive and makes the last chunk's
    fold a single 460ns op, so the post-stream critical path is short.
  - text max: running per-j-group tensor_tensor max into a [128, H] fp16
    accumulator on VectorE (max of fp16 values is exact in fp16), fp16 PE
    transposes to h-partition layout via a host-uploaded fp16 identity.
  - text sum: PE ones-matmuls accumulated in PSUM (FWL fast weight loads);
    the [1,H] PSUM sums are PE-transposed to h-partition layout and join the
    single fused cls dot (weights pre-scaled by 1/num_tokens on the host).
  - windows: host pre-gathers AND pre-masks the 32 rows around each gap
    (index/mask-shaped prep, same class as the host-gathered centers),
    uploaded split across both rings ahead of the stream; the device does
    the sum/max trees (VectorE fp16, before the first chunks land), PE
    transposes, and the ob-block folds (GpSimd, fp32).
  - final scores: per-partition dots, then one ones-matmul sums the 128
    h-partials for all 34 outputs in output order.
"""

import numpy as np
from contextlib import ExitStack

# ---- problem constants (hardcoded; harness runs kernel.py standalone) ----
B, S, H, G = 16, 4096, 768, 16
WIN = 15
WLEN = 2 * WIN + 1           # 31
NCORES = 8
EX = B // NCORES             # 2 examples per core
P = 128
JS = (4, 4, 4, 3, 2, 1)      # j-groups (128-row blocks) per stream chunk
NCH = len(JS)
JTOT = sum(JS)               # 18 j-groups = C/P rows per example
C_MIN = P * JTOT             # 2304 (capacity: valid rows per example)
OB = 4                       # 8-row blocks per (32-row padded) window
OB_R = 8                     # rows per block
NOUT = 1 + G                 # 17 scores per example

_BUILT = {}                  # capacity C -> compiled Bacc
_C = C_MIN


def _build(C):
    """Build + compile the per-core Bass program for capacity C (cached)."""
    if C in _BUILT:
        return _BUILT[C]

    import concourse.bacc as bacc
    import concourse.bass as bass
    import concourse.tile as tile
    from concourse import mybir
    from concourse.masks import make_identity

    f16 = mybir.dt.float16
    f32 = mybir.dt.float32
    AF = mybir.ActivationFunctionType
    OP = mybir.AluOpType
    AX = mybir.AxisListType

    CH = C // C_MIN  # scale factor if capacity grew (stays 1 normally)
    assert C == C_MIN * CH
    JSc = tuple(j * CH for j in JS)
    NE = EX * G              # 32

    nc = bacc.Bacc("TRN2", target_bir_lowering=False, debug=False,
                   num_devices=NCORES)

    xc_d = nc.dram_tensor("xc", [EX * C, H], f16, kind="ExternalInput").ap()
    wtp_d = nc.dram_tensor("wtp", [P, OB_R * H], f16, kind="ExternalInput").ap()
    ctr_d = nc.dram_tensor("ctr", [NE, H], f32, kind="ExternalInput").ap()
    auxf_d = nc.dram_tensor("auxf", [P, 98], f32, kind="ExternalInput").ap()
    id16_d = nc.dram_tensor("id16", [P, P], f16, kind="ExternalInput").ap()
    out_d = nc.dram_tensor("out", [EX * NOUT], f32, kind="ExternalOutput").ap()

    with tile.TileContext(nc) as tc, ExitStack() as ctx:
        singles = ctx.enter_context(tc.tile_pool(name="singles", bufs=1))
        xpool = ctx.enter_context(tc.tile_pool(name="xin", bufs=EX))
        accpool = ctx.enter_context(tc.tile_pool(name="acc", bufs=2))
        winpool = ctx.enter_context(tc.tile_pool(name="win", bufs=1))
        smalls = ctx.enter_context(tc.tile_pool(name="smalls", bufs=4))
        foldp = ctx.enter_context(tc.tile_pool(name="fold", bufs=2))
        pacc = ctx.enter_context(tc.tile_pool(name="pacc", bufs=2, space="PSUM"))
        pbig = ctx.enter_context(tc.tile_pool(name="pbig", bufs=1, space="PSUM"))
        pbigc = ctx.enter_context(tc.tile_pool(name="pbigc", bufs=1, space="PSUM"))
        psq = ctx.enter_context(tc.tile_pool(name="psq", bufs=1, space="PSUM"))
        pout = ctx.enter_context(tc.tile_pool(name="pout", bufs=1, space="PSUM"))

        # ---- ring DMAs: each HWDGE ring carries half the (pre-masked)
        # window blocks first, then one example's tapered stream chunks ----
        wtp_sb = winpool.tile([P, OB_R * H], f16)
        nc.sync.dma_start(out=wtp_sb[:, 0:4 * H], in_=wtp_d[:, 0:4 * H])
        nc.scalar.dma_start(out=wtp_sb[:, 4 * H:8 * H],
                            in_=wtp_d[:, 4 * H:8 * H])
        # ---- small aux loads ride the rings right behind the window
        # halves (tiny; SWDGE descriptor generation is too slow for these)
        ct = winpool.tile([NE, H], f32)
        nc.scalar.dma_start(out=ct[:], in_=ctr_d)
        auxf_sb = singles.tile([P, 98], f32)
        nc.sync.dma_start(out=auxf_sb[:], in_=auxf_d)
        id16 = singles.tile([P, P], f16)
        nc.sync.dma_start(out=id16[:], in_=id16_d)

        xts = {}
        for T in range(NCH):
            for ex in range(EX):
                j = JSc[T]
                xt = xpool.tile([P, j * H], f16, name=f"xt{ex}_{T}",
                                tag=f"xt{T}")
                row0 = ex * C + sum(JSc[:T]) * P
                src = bass.AP(xc_d.tensor, row0 * H, [[j * H, P], [1, j * H]])
                eng = nc.sync if ex == 0 else nc.scalar
                eng.dma_start(out=xt[:], in_=src)
                xts[(ex, T)] = xt

        # ---- stream compute pieces ----
        accs = [accpool.tile([P, H], f16, name=f"acc{e}", tag="acc")
                for e in range(EX)]
        pss = [pacc.tile([1, H], f32, name=f"ps{e}", tag="ps")
               for e in range(EX)]

        # PE p-state warmup: the PE idles until the first stream chunk
        # lands; ~12 dummy matmuls keep it executing so it ramps to full
        # clock before the real accumulation starts. They write the ps
        # banks, which the real accumulation group resets via start=True.
        wsc = singles.tile([P, 512], f16)
        nc.vector.memset(wsc[:], 0.0)
        ones16 = singles.tile([P, 1], f16)
        nc.vector.memset(ones16[:], 1.0)
        for i in range(12):
            nc.tensor.matmul(out=pss[i % EX][0:1, 0:512], lhsT=ones16[:],
                             rhs=wsc[:], start=True, stop=True)

        ident = singles.tile([P, P], f32)
        make_identity(nc, ident[:])
        ones = singles.tile([P, 1], f32)
        nc.vector.memset(ones[:], 1.0)
        # one-time ACT table load, after the scalar-ring DMAs are issued
        warm = singles.tile([1, 1], f32)
        nc.scalar.activation(out=warm[:], in_=ones[0:1, 0:1], func=AF.Copy)

        pooled_a = auxf_sb[:, 0:12]          # 2ex x 6
        cwc_a = auxf_sb[:, 12:48]            # 2ex x 18 (max | pooled | sum/tn)
        invcnt_a = auxf_sb[:, 48:80]         # 32
        gwt_a = auxf_sb[:, 80:98]            # 18

        # rhs of the final ones-matmul, in output order:
        # col ex*17 = cls partials, cols ex*17+1+g = gap partials
        rhs34 = smalls.tile([P, EX * NOUT], f32)

        def stream_mm(ex, T):
            xt = xts[(ex, T)]
            ps = pss[ex]
            for j in range(JSc[T]):
                first = (T == 0 and j == 0)
                last = (T == NCH - 1 and j == JSc[T] - 1)
                nc.tensor.matmul(out=ps[0:1, 0:512],
                                 lhsT=ones16[:],
                                 rhs=xt[:, j * H:j * H + 512],
                                 start=first, stop=last)
                nc.tensor.matmul(out=ps[0:1, 512:H],
                                 lhsT=ones16[:],
                                 rhs=xt[:, j * H + 512:(j + 1) * H],
                                 start=first, stop=last)

        msc = accpool.tile([P, 2 * H], f16, name="msc", tag="msc")

        def stream_max(ex, T):
            # fold the chunk's j-groups into the [P, H] accumulator; use a
            # halving tree for power-of-two group counts (fewer, wider TTs)
            xt = xts[(ex, T)]
            acc = accs[ex]
            n = JSc[T]
            if n % 2 == 0:
                while n > 1:
                    n //= 2
                    src_ = xt if n * 2 == JSc[T] else msc
                    dst = msc
                    nc.vector.tensor_tensor(out=dst[:, 0:n * H],
                                            in0=src_[:, 0:n * H],
                                            in1=src_[:, n * H:2 * n * H],
                                            op=OP.max)
                if T == 0:
                    nc.vector.tensor_copy(out=acc[:], in_=msc[:, 0:H])
                else:
                    nc.vector.tensor_tensor(out=acc[:], in0=acc[:],
                                            in1=msc[:, 0:H], op=OP.max)
            else:
                for j in range(JSc[T]):
                    in0 = xt[:, 0:H] if (T == 0 and j == 1) else acc[:]
                    if T == 0 and j == 0:
                        continue
                    nc.vector.tensor_tensor(out=acc[:], in0=in0,
                                            in1=xt[:, j * H:(j + 1) * H],
                                            op=OP.max)

        ws = winpool.tile([P, OB_R * H // 2], f16)
        wsF = winpool.tile([P, H], f16)
        wm = winpool.tile([P, OB_R * H // 2], f16)
        wtF = winpool.tile([P, H], f16)
        gfeat = winpool.tile([P, 3 * 6 * NE], f32)       # [cT|maxT|sumT]
        gfold = winpool.tile([P, 2 * 6 * NE], f32)

        def emit_window_a():
            # sum/max trees on VectorE fp16 (rows pre-mask-zeroed on host)
            nc.vector.tensor_tensor(out=ws[:], in0=wtp_sb[:, 0:4 * H],
                                    in1=wtp_sb[:, 4 * H:8 * H], op=OP.add)
            nc.vector.tensor_tensor(out=ws[:, 0:2 * H], in0=ws[:, 0:2 * H],
                                    in1=ws[:, 2 * H:4 * H], op=OP.add)
            nc.vector.tensor_tensor(out=wsF[:], in0=ws[:, 0:H],
                                    in1=ws[:, H:2 * H], op=OP.add)
            nc.vector.tensor_tensor(out=wm[:], in0=wtp_sb[:, 0:4 * H],
                                    in1=wtp_sb[:, 4 * H:8 * H], op=OP.max)
            nc.vector.tensor_tensor(out=wm[:, 0:2 * H], in0=wm[:, 0:2 * H],
                                    in1=wm[:, 2 * H:4 * H], op=OP.max)
            nc.vector.tensor_tensor(out=wtF[:], in0=wm[:, 0:H],
                                    in1=wm[:, H:2 * H], op=OP.max)
            # centers can be transposed as soon as ct + ident are loaded
            ptC = pbigc.tile([P, 6 * NE], f32)
            for c in range(6):
                nc.tensor.transpose(out=ptC[:, c * NE:(c + 1) * NE],
                                    in_=ct[:, c * P:(c + 1) * P],
                                    identity=ident[0:NE, 0:NE])
            nc.scalar.activation(out=gfeat[:, 0:6 * NE], in_=ptC[:],
                                 func=AF.Copy)

        def emit_window_b():
            # transpose to h-partition layout, copy PSUM->SBUF on ScalarE,
            # fold the 4 ob-groups with cheap TT ops on GpSimd (fp32)
            def obfold(gm, dst, op, eng):
                # gm free layout: c*128 + ob*32 + e (c in 6, ob in 4, e in 32)
                g = gm[:]
                v = [bass.AP(g.tensor, g.offset + ob * NE,
                             [g.ap[0], [P, 6], [1, NE]]) for ob in range(OB)]
                f = gfold[:]
                f01 = bass.AP(f.tensor, f.offset, [f.ap[0], [NE, 6], [1, NE]])
                f23 = bass.AP(f.tensor, f.offset + 6 * NE,
                              [f.ap[0], [NE, 6], [1, NE]])
                d = bass.AP(dst.tensor, dst.offset,
                            [dst.ap[0], [NE, 6], [1, NE]])
                eng.tensor_tensor(out=f01, in0=v[0], in1=v[1], op=op)
                eng.tensor_tensor(out=f23, in0=v[2], in1=v[3], op=op)
                eng.tensor_tensor(out=d, in0=f01, in1=f23, op=op)

            ptM = pbig.tile([P, H], f16, tag="ptw")
            for c in range(6):
                nc.tensor.transpose(out=ptM[:, c * P:(c + 1) * P],
                                    in_=wtF[:, c * P:(c + 1) * P],
                                    identity=id16[:])
            gmM = winpool.tile([P, H], f32)
            nc.scalar.activation(out=gmM[:], in_=ptM[:], func=AF.Copy)
            obfold(gmM, gfeat[:, 6 * NE:12 * NE], OP.max, nc.vector)
            nc.vector.tensor_scalar_max(out=gfeat[:, 6 * NE:12 * NE],
                                        in0=gfeat[:, 6 * NE:12 * NE],
                                        scalar1=0.0)
            ptS = pbig.tile([P, H], f16, tag="ptw")
            for c in range(6):
                nc.tensor.transpose(out=ptS[:, c * P:(c + 1) * P],
                                    in_=wsF[:, c * P:(c + 1) * P],
                                    identity=id16[:])
            gmS = winpool.tile([P, H], f32)
            nc.scalar.activation(out=gmS[:], in_=ptS[:], func=AF.Copy)
            obfold(gmS, gfeat[:, 12 * NE:18 * NE], OP.add, nc.vector)
            # avg = sum / cnt  (per (ex,g) along free)
            icnt_b = bass.AP(invcnt_a.tensor, invcnt_a.offset,
                             [invcnt_a.ap[0], [0, 6], [1, NE]])
            gf_s = bass.AP(gfeat[:].tensor, gfeat[:].offset + 12 * NE,
                           [gfeat[:].ap[0], [NE, 6], [1, NE]])
            nc.vector.tensor_tensor(out=gf_s, in0=gf_s, in1=icnt_b,
                                    op=OP.mult)

            # combined gap dot: gfeat[p,(part,c,exg)] * W[part*H + c*128 + p]
            gw_b = bass.AP(gwt_a.tensor, gwt_a.offset,
                           [gwt_a.ap[0], [6, 3], [1, 6], [0, NE]])
            gf_v = bass.AP(gfeat[:].tensor, gfeat[:].offset,
                           [gfeat[:].ap[0], [6 * NE, 3], [NE, 6], [1, NE]])
            nc.vector.tensor_tensor(out=gf_v, in0=gf_v, in1=gw_b, op=OP.mult)
            gf_r = bass.AP(gfeat[:].tensor, gfeat[:].offset,
                           [gfeat[:].ap[0], [1, NE], [NE, 18]])
            rhs_g = bass.AP(rhs34[:].tensor, rhs34[:].offset + 1,
                            [rhs34[:].ap[0], [NOUT, EX], [1, G]])
            nc.vector.tensor_reduce(out=rhs_g, in_=gf_r, axis=AX.X,
                                    op=OP.add)

        def finalize_ex(ex):
            maxf = accs[ex]
            pt = pbig.tile([P, H], f16, tag="ptw")
            for c in range(6):
                nc.tensor.transpose(out=pt[:, c * P:(c + 1) * P],
                                    in_=maxf[:, c * P:(c + 1) * P],
                                    identity=id16[:])
            ptsb = foldp.tile([P, H], f16)
            nc.scalar.activation(out=ptsb[:], in_=pt[:], func=AF.Copy)
            feat6 = foldp.tile([P, 6], f32)
            pt_v = ptsb[:].rearrange("p (c s) -> p c s", c=6)
            nc.vector.tensor_reduce(out=feat6[:], in_=pt_v, axis=AX.X,
                                    op=OP.max)
            # zero-padding may be absent (nv == C): floor at 0 here
            nc.vector.tensor_scalar_max(out=feat6[:], in0=feat6[:],
                                        scalar1=0.0)

            # text-sum: PSUM [1,H] -> SBUF -> PE transpose to h-partitions
            ps = pss[ex]
            pssb = foldp.tile([1, H], f32, name=f"pssb{ex}", tag="pssb")
            nc.scalar.activation(out=pssb[:], in_=ps[:], func=AF.Copy)
            ptq = psq.tile([P, 6], f32, name=f"ptq{ex}", tag="ptq")
            for c in range(6):
                nc.tensor.transpose(out=ptq[:, c:c + 1],
                                    in_=pssb[0:1, c * P:(c + 1) * P],
                                    identity=ident[0:1, 0:1])
            sum6 = foldp.tile([P, 6], f32, name=f"sum6{ex}", tag="sum6")
            nc.scalar.activation(out=sum6[:], in_=ptq[:], func=AF.Copy)

            # cls partials: [text-max | pooled | text-sum/tn] . cls_W
            cprod = foldp.tile([P, 18], f32)
            nc.vector.tensor_tensor(out=cprod[:, 0:6], in0=feat6[:],
                                    in1=cwc_a[:, ex * 18:ex * 18 + 6],
                                    op=OP.mult)
            nc.vector.tensor_tensor(out=cprod[:, 6:12],
                                    in0=pooled_a[:, ex * 6:(ex + 1) * 6],
                                    in1=cwc_a[:, ex * 18 + 6:ex * 18 + 12],
                                    op=OP.mult)
            nc.vector.tensor_tensor(out=cprod[:, 12:18], in0=sum6[:],
                                    in1=cwc_a[:, ex * 18 + 12:ex * 18 + 18],
                                    op=OP.mult)
            cidx = ex * NOUT
            nc.vector.tensor_reduce(out=rhs34[:, cidx:cidx + 1],
                                    in_=cprod[:], axis=AX.X, op=OP.add)

        # ---- emission order: window head early (fills VectorE before the
        # stream lands), then per-chunk compute as chunks arrive, window
        # tail mid-stream, per-example finalization, output ----
        emit_window_a()
        for T in range(NCH):
            for ex in range(EX):
                stream_mm(ex, T)
            for ex in range(EX):
                stream_max(ex, T)
            if T == 2:
                emit_window_b()
        for ex in range(EX):
            finalize_ex(ex)

        # ---- final ones-matmul (sums partials over h' partitions) ----
        pscore = pout.tile([1, EX * NOUT], f32)
        nc.tensor.matmul(out=pscore[:], lhsT=ones[:], rhs=rhs34[:],
                         start=True, stop=True)
        sg = smalls.tile([1, EX * NOUT], f32)
        nc.scalar.activation(out=sg[:], in_=pscore[:], func=AF.Copy)
        nc.sync.dma_start(out=out_d[:], in_=sg[0:1, :])

    nc.compile()
    _BUILT[C] = nc
    return nc


def _prep_core(seq_c, pooled_c, bm_c, gids_c, gW, cW, C):
    """Host-side per-core input prep. seq_c [EX,S,H] f32 (view), bm_c [EX,S]
    bool, gids_c [EX,G] int, gW [3H] f32, cW [3H] f32, C = capacity."""
    f32 = np.float32
    f16 = np.float16

    xc = np.zeros((EX * C, H), dtype=f16)
    a = np.empty((EX, G), dtype=np.int64)
    b = np.empty((EX, G), dtype=np.int64)
    tn = np.empty((EX,), dtype=f32)
    for ex in range(EX):
        pos = np.flatnonzero(bm_c[ex])
        nv = len(pos)
        tn[ex] = nv
        xc[ex * C:ex * C + nv] = seq_c[ex, pos].astype(f16)
        a[ex] = np.searchsorted(pos, gids_c[ex] - WIN, side="left")
        b[ex] = np.searchsorted(pos, gids_c[ex] + WIN, side="right")

    # window partitions: p = ob*32 + ex*16 + g; each holds OB_R=8 compacted
    # rows starting at row start + ob*8 of a 32-row padded block, pre-masked
    NE = EX * G
    obv = np.repeat(np.arange(OB), NE)            # [P]
    exv = np.tile(np.repeat(np.arange(EX), G), OB)
    gv = np.tile(np.arange(G), EX * OB)
    a_p = a[exv, gv]                              # [P]
    b_p = b[exv, gv]
    start = np.clip(a_p, 0, C - OB * OB_R)        # [P] padded-block start
    rows = (start + obv * OB_R)[:, None] + np.arange(OB_R)[None, :]  # [P, 8]
    wmask = ((rows >= a_p[:, None]) & (rows < b_p[:, None]))
    gath = xc[(exv * C)[:, None] + rows]          # [P, 8, H] f16
    wtp = (gath * wmask[:, :, None].astype(f16)).reshape(P, OB_R * H)

    cnt = (b - a).astype(f32)                     # [EX, G]
    with np.errstate(divide="ignore"):
        icnt = 1.0 / cnt

    exg_e = np.repeat(np.arange(EX), G)
    exg_g = np.tile(np.arange(G), EX)
    ctr = np.ascontiguousarray(
        seq_c[exg_e, gids_c[exg_e, exg_g]], dtype=f32)     # [NE, H]

    # auxf[:, 0:12] pooledr, [:, 12:48] cwc (max|pooled|sum/tn per ex),
    # [:, 48:80] invcnt, [:, 80:98] gwt
    auxf = np.empty((P, 98), f32)
    cw3 = cW.reshape(3, 6, P)                     # [part, c, p]
    for ex in range(EX):
        auxf[:, ex * 6:(ex + 1) * 6] = pooled_c[ex].reshape(6, P).T
        o = 12 + ex * 18
        auxf[:, o:o + 6] = cw3[1].T               # text-max weights
        auxf[:, o + 6:o + 12] = cw3[0].T          # pooled weights
        auxf[:, o + 12:o + 18] = cw3[2].T / tn[ex]  # text-sum weights / tn
    auxf[:, 48:80] = np.broadcast_to(icnt.reshape(NE), (P, NE))
    auxf[:, 80:98] = gW.reshape(3, 6, P).transpose(2, 0, 1).reshape(P, 18)

    id16 = np.eye(P, dtype=f16)

    return {
        "xc": xc,
        "wtp": wtp,
        "ctr": ctr,
        "auxf": auxf,
        "id16": id16,
    }


def _make_in_maps(sequence_output, pooled_output, token_type_ids, word_mask,
                  gap_ids, gap_W, cls_W):
    global _C
    seq = np.asarray(sequence_output, dtype=np.float32)
    pooled = np.asarray(pooled_output, dtype=np.float32)
    tti = np.asarray(token_type_ids)
    wmk = np.asarray(word_mask)
    gids = np.asarray(gap_ids).astype(np.int64)
    gW = np.asarray(gap_W, dtype=np.float32)
    cW = np.asarray(cls_W, dtype=np.float32)
    base_mask = (tti == 0) & (wmk != 0)

    max_nv = int(base_mask.sum(axis=1).max())
    C = max(C_MIN, -(-max_nv // C_MIN) * C_MIN)
    # keep the compiled capacity if it still fits (avoid rebuilds)
    if _BUILT and any(c >= C for c in _BUILT):
        C = min(c for c in _BUILT if c >= C)
    _C = C

    in_maps = []
    for c in range(NCORES):
        lo = c * EX
        in_maps.append(_prep_core(seq[lo:lo + EX], pooled[lo:lo + EX],
                                  base_mask[lo:lo + EX], gids[lo:lo + EX],
                                  gW, cW, C))
    return in_maps


def _run(in_maps, trace=False, trace_cores=None):
    from concourse import bass_utils
    nc = _build(_C)
    return bass_utils.run_bass_kernel_spmd(
        nc, in_maps, core_ids=list(range(NCORES)), trace=trace,
        trace_cores=trace_cores)


def kernel(sequence_output, pooled_output, token_type_ids, word_mask,
           gap_ids, gap_W, gap_b, cls_W, cls_b):
    in_maps = _make_in_maps(sequence_output, pooled_output, token_type_ids,
                            word_mask, gap_ids, gap_W, cls_W)
    res = _run(in_maps)
    out = np.concatenate(
        [res.results[c]["out"].reshape(EX, NOUT) for c in range(NCORES)], 0)
    out[:, 0] += float(np.asarray(cls_b))
    out[:, 1:] += float(np.asarray(gap_b))
    return out.astype(np.float32)
